# revision 16
# baseline (speedup 1.0000x reference)
"""GatNet on 8 Trainium2 NeuronCores (Bass/Tile).

4-layer GAT (8 heads) + mean/max graph pooling + FC + log_softmax.

Sharding: nodes are partitioned across the 8 cores by destination (each core
owns ~12.5k dst nodes and all their incoming edges).  Within a core, nodes are
degree-sorted and processed in 128-node groups; each group's incoming-edge
source rows are fetched with one indirect DMA gather from a replicated
[h | a_src.h] table in HBM.  The attention softmax and weighted aggregation
are segment-free: each dst node owns one SBUF partition, its edges live along
the free dimension (padded to the group max degree), so segment reductions
are plain free-dim reductions.  Layer GEMMs are sharded per-core; the
resulting feature tables are exchanged with chunked AllGather collectives that
overlap the edge phase.

Host-side work is limited to index/layout preprocessing (permutations, gather
indices, padding masks) and weight folding (a_src/a_dst contracted into the
layer weight) - all model FLOPs on [N]/[E]-sized data run on device.
"""

import os

import numpy as np

H = 8
N_NODES = 100000
N_EDGES = 1600000
N_GRAPHS = 512
NC = 8
P = 128
# (Fin, Cout) per GAT layer
LAYERS = [(18, 64), (64, 128), (128, 128), (128, 128)]

LAST_EXEC_NS = None  # set by kernel() when GAT_TRACE=1

ALS_NEG = -10000.0  # a_src logit of the padding row -> exp() == 0
SLOT_CAP = 96  # max Q*D slots per edge-phase batch (SBUF budget)
QMAX = 4
DGC = 64  # pooling gather chunk (slots per chunk)


# ---------------------------------------------------------------------------
# host-side preprocessing (indices / layout only)
# ---------------------------------------------------------------------------

class _Cfg:
    pass


def _prep(x, edge_index, batch, n_nodes, n_graphs, nc_cores=NC, ch=None):
    cfg = _Cfg()
    n = n_nodes
    npc_raw = n // nc_cores
    assert npc_raw * nc_cores == n
    NPC = ((npc_raw + P - 1) // P) * P
    groups = NPC // P
    if ch is None:
        ch = 7 if groups % 7 == 0 else (2 if groups % 2 == 0 else 1)
    assert groups % ch == 0
    R = NPC // ch
    NP_all = nc_cores * NPC
    PAD_ROW = NP_all

    src = np.concatenate([np.asarray(edge_index[0]), np.arange(n, dtype=np.int64)])
    dst = np.concatenate([np.asarray(edge_index[1]), np.arange(n, dtype=np.int64)])
    deg = np.bincount(dst, minlength=n)

    # per-core node sets and degree-sorted local permutation
    perm_local = np.empty((nc_cores, npc_raw), dtype=np.int64)
    for c in range(nc_cores):
        nodes = np.arange(c * npc_raw, (c + 1) * npc_raw)
        order = np.argsort(-deg[nodes], kind="stable")
        perm_local[c] = nodes[order]

    # pos_of: original node id -> row in the AllGather-chunked tables
    pos_of = np.empty(n, dtype=np.int64)
    lpos_all = np.arange(npc_raw)
    for c in range(nc_cores):
        k = lpos_all // R
        pos = k * (nc_cores * R) + c * R + (lpos_all % R)
        pos_of[perm_local[c]] = pos

    # incoming edge lists in dst order
    e_order = np.argsort(dst, kind="stable")
    src_s = src[e_order]
    starts = np.zeros(n + 1, dtype=np.int64)
    np.cumsum(deg, out=starts[1:])

    # group max degrees (shared across cores for SPMD shape uniformity)
    deg_l = np.zeros((nc_cores, NPC), dtype=np.int64)
    for c in range(nc_cores):
        deg_l[c, :npc_raw] = deg[perm_local[c]]
    Dg = np.zeros(groups, dtype=np.int64)
    for g in range(groups):
        Dg[g] = deg_l[:, g * P:(g + 1) * P].max()
    Dg = np.maximum(Dg, 1)

    # batches of Q groups sharing a slot count, not crossing chunk boundaries
    gpc = groups // ch
    batches = []  # (g0, Q, D)
    g = 0
    while g < groups:
        D = int(Dg[g])
        q = 1
        while (q < QMAX and g + q < groups
               and (q + 1) * D <= SLOT_CAP
               and (g // gpc) == ((g + q) // gpc)):
            q += 1
        batches.append((g, q, D))
        g += q
    cfg.batches = batches
    cfg.QD_max = max(q * d for _, q, d in batches)

    # per-core edge gather indices (positions into the feature tables)
    tot_slots = sum(P * q * d for _, q, d in batches)
    eidx = np.full((nc_cores, tot_slots), PAD_ROW, dtype=np.int32)
    for c in range(nc_cores):
        off = 0
        for (g0, q, D) in batches:
            blk = np.full((P, q, D), PAD_ROW, dtype=np.int32)
            for qi in range(q):
                base = (g0 + qi) * P
                for p in range(P):
                    lp = base + p
                    if lp >= npc_raw:
                        continue
                    node = perm_local[c, lp]
                    s0, s1 = starts[node], starts[node + 1]
                    blk[p, qi, :s1 - s0] = pos_of[src_s[s0:s1]]
            sz = P * q * D
            eidx[c, off:off + sz] = blk.reshape(P, q * D).ravel()
            off += sz
        assert off == tot_slots

    # pooling layout: graphs per core, chunked gathers of DGC slots
    ngc = n_graphs // nc_cores
    batch = np.asarray(batch)
    gstarts = np.searchsorted(batch, np.arange(n_graphs + 1))
    sizes = np.diff(gstarts)
    sg_max = max(int(sizes.max()), 1)
    nch_g = (sg_max + DGC - 1) // DGC
    DGP = nch_g * DGC
    gidx = np.zeros((nc_cores, ngc, DGP), dtype=np.int32)
    gmask = np.zeros((nc_cores, ngc, DGP), dtype=np.float32)
    ginv = np.zeros((nc_cores, ngc, 1), dtype=np.float32)
    gmax = np.zeros((nc_cores, ngc, 1), dtype=np.float32)
    for c in range(nc_cores):
        for i in range(ngc):
            gid = c * ngc + i
            nodes = np.arange(gstarts[gid], gstarts[gid + 1])
            sz = len(nodes)
            if sz == 0:
                continue
            rows = pos_of[nodes]
            gidx[c, i, :sz] = rows
            gidx[c, i, sz:] = rows[0]
            gmask[c, i, :sz] = 1.0
            ginv[c, i, 0] = 1.0 / sz
            gmax[c, i, 0] = 1.0

    cfg.n = n
    cfg.nc_cores = nc_cores
    cfg.npc_raw = npc_raw
    cfg.NPC = NPC
    cfg.groups = groups
    cfg.CH = ch
    cfg.R = R
    cfg.NP_all = NP_all
    cfg.PAD_ROW = PAD_ROW
    cfg.tot_slots = tot_slots
    cfg.ngc = ngc
    cfg.DGP = DGP
    cfg.nch_g = nch_g
    cfg.n_graphs = n_graphs
    cfg.perm_local = perm_local
    cfg.pos_of = pos_of
    cfg.eidx = eidx
    cfg.gidx = gidx
    cfg.gmask = gmask
    cfg.ginv = ginv
    cfg.gmax = gmax
    return cfg


def _fold_weights(W, a_s, a_d, fin, cout):
    """W' = [W | W@a_src per head | W@a_dst per head]  ->  [Fin, Cout+16]."""
    W = np.asarray(W, dtype=np.float32)
    a_s = np.asarray(a_s, dtype=np.float32)
    a_d = np.asarray(a_d, dtype=np.float32)
    c = cout // H
    Wh = W.reshape(fin, H, c)
    w_as = np.einsum("fhc,hc->fh", Wh, a_s)
    w_ad = np.einsum("fhc,hc->fh", Wh, a_d)
    return np.concatenate([W, w_as, w_ad], axis=1)


# ---------------------------------------------------------------------------
# bass program
# ---------------------------------------------------------------------------

def _build(cfg):
    from contextlib import ExitStack
    import concourse.tile as tile
    from concourse import bacc, mybir
    from concourse.bass import AP, IndirectOffsetOnAxis
    from concourse.masks import make_identity

    bf16 = mybir.dt.bfloat16
    f32 = mybir.dt.float32
    i32 = mybir.dt.int32
    AF = mybir.ActivationFunctionType
    OP = mybir.AluOpType
    AX = mybir.AxisListType

    nc = bacc.Bacc("TRN2", target_bir_lowering=False, debug=False,
                   num_devices=cfg.nc_cores)

    NPC, NP_all, CH, R = cfg.NPC, cfg.NP_all, cfg.CH, cfg.R
    groups = cfg.groups
    gpc = groups // CH  # groups per collective chunk
    ngc, DGP = cfg.ngc, cfg.DGP
    replica = [list(range(cfg.nc_cores))]
    QDM = cfg.QD_max
    HS_MAX = LAYERS[-1][1] + 8  # widest gather row (136)

    # ---- I/O ----
    xT1f = nc.dram_tensor("xT1_full", [LAYERS[0][0], NP_all], bf16, kind="ExternalInput")
    xT1o = nc.dram_tensor("xT1_own", [LAYERS[0][0], NPC], bf16, kind="ExternalInput")
    Wp, Bs = [], []
    for li, (fin, cout) in enumerate(LAYERS):
        Wp.append(nc.dram_tensor(f"W{li+1}p", [fin, cout + 16], bf16, kind="ExternalInput"))
        Bs.append(nc.dram_tensor(f"b{li+1}", [P, cout], f32, kind="ExternalInput"))
    fcW = nc.dram_tensor("fcW", [256, 6], bf16, kind="ExternalInput")
    fcb = nc.dram_tensor("fcb", [ngc, 6], f32, kind="ExternalInput")
    eidx = nc.dram_tensor("eidx", [1, cfg.tot_slots], i32, kind="ExternalInput")
    gidx = nc.dram_tensor("gidx", [1, ngc * DGP], i32, kind="ExternalInput")
    gmask = nc.dram_tensor("gmask", [1, ngc * DGP], bf16, kind="ExternalInput")
    ginv = nc.dram_tensor("ginv", [ngc, 1], f32, kind="ExternalInput")
    gmax = nc.dram_tensor("gmax", [ngc, 1], f32, kind="ExternalInput")
    outp = nc.dram_tensor("out", [ngc, 6], f32, kind="ExternalOutput")

    # ---- internal DRAM ----
    hs1_t = nc.dram_tensor("hs1_t", [NP_all + 1, 72], bf16)
    hs_t = [nc.dram_tensor(f"hs_t{i}", [NP_all + 1, 136], bf16,
                           addr_space="Shared") for i in range(2)]
    hs_sl = [nc.dram_tensor(f"hs_sl{k}", [R, 136], bf16) for k in range(CH)]
    x4_sl = [nc.dram_tensor(f"x4_sl{k}", [R, 128], bf16) for k in range(CH)]
    x4_t = nc.dram_tensor("x4_t", [NP_all, 128], bf16, addr_space="Shared")

    def dram_ap(t, row0, nrow, ncol):
        """[p, i, c] view of DRAM rows row0 + i*P + p."""
        w = t.shape[1]
        return AP(t, row0 * w, [[w, P], [P * w, nrow // P], [1, ncol]])

    def ps_(t):
        return t.ap[0][0]  # partition free-stride of a pool-tile AP

    def vap(t, off, dims):
        return AP(t.tensor, t.offset + off, dims)

    with tile.TileContext(nc) as tc, ExitStack() as ctx:
        const = ctx.enter_context(tc.tile_pool(name="const", bufs=1))
        xt_pool = ctx.enter_context(tc.tile_pool(name="xt", bufs=2))
        ald_pool = ctx.enter_context(tc.tile_pool(name="ald", bufs=1))
        mm_psum = ctx.enter_context(tc.tile_pool(name="mmps", bufs=4, space="PSUM"))
        tp_psum = ctx.enter_context(tc.tile_pool(name="tpps", bufs=3, space="PSUM"))
        hs_pool = ctx.enter_context(tc.tile_pool(name="hssb", bufs=2))
        idx_pool = ctx.enter_context(tc.tile_pool(name="idx", bufs=3))
        gath_pool = ctx.enter_context(tc.tile_pool(name="gath", bufs=2))
        pg_pool = ctx.enter_context(tc.tile_pool(name="pgath", bufs=1))
        e_pool = ctx.enter_context(tc.tile_pool(name="epool", bufs=3))
        sm_pool = ctx.enter_context(tc.tile_pool(name="small", bufs=3))
        out_pool = ctx.enter_context(tc.tile_pool(name="outp", bufs=2))

        # ---- constants ----
        ident = const.tile([P, P], bf16)
        make_identity(nc, ident[:])
        W_sb, B_sb = [], []
        for li, (fin, cout) in enumerate(LAYERS):
            w = const.tile([fin, cout + 16], bf16, tag=f"w{li}")
            nc.sync.dma_start(out=w[:], in_=Wp[li].ap())
            W_sb.append(w)
            b = const.tile([P, cout], f32, tag=f"b{li}")
            nc.sync.dma_start(out=b[:], in_=Bs[li].ap())
            B_sb.append(b)
        fcW_sb = const.tile([P, 2, 6], bf16)
        nc.sync.dma_start(out=fcW_sb[:], in_=AP(fcW, 0, [[6, P], [P * 6, 2], [1, 6]]))
        fcb_sb = const.tile([ngc, 6], f32)
        nc.sync.dma_start(out=fcb_sb[:], in_=fcb.ap())
        ginv_sb = const.tile([ngc, 1], f32)
        nc.sync.dma_start(out=ginv_sb[:], in_=ginv.ap())
        gmax_sb = const.tile([ngc, 1], f32)
        nc.sync.dma_start(out=gmax_sb[:], in_=gmax.ap())
        xT1o_sb = const.tile([LAYERS[0][0], NPC], bf16)
        nc.sync.dma_start(out=xT1o_sb[:], in_=xT1o.ap())

        # padding rows of the gather tables: h = 0, a_src logit = ALS_NEG
        padrow = const.tile([1, 136], bf16)
        nc.vector.memset(padrow[:, 0:128], 0.0)
        nc.vector.memset(padrow[:, 128:136], ALS_NEG)
        nc.sync.dma_start(out=AP(hs1_t, NP_all * 72, [[72, 1], [1, 64]]),
                          in_=padrow[:, 0:64])
        nc.sync.dma_start(out=AP(hs1_t, NP_all * 72 + 64, [[72, 1], [1, 8]]),
                          in_=padrow[:, 128:136])
        for t in hs_t:
            nc.sync.dma_start(out=AP(t, NP_all * 136, [[136, 1], [1, 136]]),
                              in_=padrow[:])

        # ---- layer-1 GEMM, replicated over all nodes ----
        blocks_all = NP_all // P
        fin1, cout1 = LAYERS[0]
        step1 = 7
        for b0 in range(0, blocks_all, step1):
            nb = min(step1, blocks_all - b0)
            big = hs_pool.tile([P, step1, 72], bf16, tag="hs1big")
            xch = idx_pool.tile([LAYERS[0][0], step1 * P], bf16, tag="xch")
            nc.sync.dma_start(out=xch[:, 0:nb * P],
                              in_=xT1f.ap()[:, b0 * P:(b0 + nb) * P])
            for i in range(nb):
                ps = mm_psum.tile([P, 144], f32, tag="ps")
                nc.tensor.matmul(out=ps[:, 0:cout1 + 8],
                                 lhsT=xch[:, i * P:(i + 1) * P],
                                 rhs=W_sb[0][:, 0:cout1 + 8], start=True, stop=True)
                if i % 2 == 0:
                    nc.scalar.copy(out=big[:, i, :], in_=ps[:, 0:72])
                else:
                    nc.vector.tensor_copy(out=big[:, i, :], in_=ps[:, 0:72])
            nc.sync.dma_start(out=dram_ap(hs1_t, b0 * P, nb * P, 72),
                              in_=big[:, 0:nb, :])

        # layer-1 a_dst logits for own nodes
        ald_sb = [ald_pool.tile([P, groups * 8], bf16, tag=f"ald{i}",
                                name=f"ald{i}")
                  for i in range(2)]
        for g in range(groups):
            ps = mm_psum.tile([P, 144], f32, tag="ps")
            nc.tensor.matmul(out=ps[:, 0:16],
                             lhsT=xT1o_sb[:, g * P:(g + 1) * P],
                             rhs=W_sb[0][:, cout1:cout1 + 16],
                             start=True, stop=True)
            nc.vector.tensor_copy(out=ald_sb[0][:, g * 8:(g + 1) * 8],
                                  in_=ps[:, 8:16])

        # ---- layers ----
        for li in range(4):
            fin, cout = LAYERS[li]
            cpl = cout // H  # channels per head (8 or 16)
            hs_cols = cout + 8
            table = hs1_t if li == 0 else hs_t[(li - 1) % 2]
            tab_ap = table.ap()
            ald_cur = ald_sb[li % 2]
            ald_nxt = ald_sb[(li + 1) % 2]
            last = li == 3
            if not last:
                cout_n = LAYERS[li + 1][1]
                xT_next = xt_pool.tile([P, NPC], bf16, tag="xt")
                ntab = hs_t[li % 2]
            eoff = 0
            hs_big = None
            for (g0, Q, D) in cfg.batches:
                QD = Q * D
                # gather indices + rows
                it = idx_pool.tile([P, QDM], i32, tag="idx")
                nc.sync.dma_start(out=it[:, 0:QD],
                                  in_=AP(eidx, eoff, [[QD, P], [1, QD]]))
                eoff += P * QD
                gt = gath_pool.tile([P, QDM * HS_MAX], bf16, tag="gath")
                gps = ps_(gt)
                for j in range(QD):
                    nc.gpsimd.indirect_dma_start(
                        out=vap(gt, j * hs_cols, [[gps, P], [1, hs_cols]]),
                        out_offset=None, in_=tab_ap,
                        in_offset=IndirectOffsetOnAxis(ap=it[:, j:j + 1], axis=0))

                gals = vap(gt, cout, [[gps, P], [hs_cols * D, Q], [hs_cols, D], [1, 8]])

                # e = lrelu(a_src[src] + a_dst[dst]);  w = exp(e)
                et = e_pool.tile([P, QDM * 8], bf16, tag="et")
                e_v = vap(et, 0, [[ps_(et), P], [D * 8, Q], [8, D], [1, 8]])
                ald_v = vap(ald_cur, g0 * 8, [[ps_(ald_cur), P], [8, Q], [0, D], [1, 8]])
                nc.vector.tensor_tensor(out=e_v, in0=gals, in1=ald_v, op=OP.add)
                wt = e_pool.tile([P, QDM * 8], bf16, tag="wt")
                w_v = vap(wt, 0, [[ps_(wt), P], [D * 8, Q], [8, D], [1, 8]])
                nc.vector.tensor_scalar(out=w_v, in0=e_v, scalar1=0.2,
                                        scalar2=None, op0=OP.mult)
                nc.vector.tensor_tensor(out=e_v, in0=e_v, in1=w_v, op=OP.max)
                nc.scalar.activation(out=w_v, in_=e_v, func=AF.Exp)

                # denom + reciprocal
                den = sm_pool.tile([P, QMAX * 8], f32, tag="den")
                den_v = vap(den, 0, [[ps_(den), P], [8, Q], [1, 8]])
                w_hd = vap(wt, 0, [[ps_(wt), P], [D * 8, Q], [1, 8], [8, D]])
                nc.vector.tensor_reduce(out=den_v, in_=w_hd, axis=AX.X, op=OP.add)
                nc.vector.tensor_scalar(out=den_v, in0=den_v, scalar1=1e-30,
                                        scalar2=None, op0=OP.add)
                rec = sm_pool.tile([P, QMAX * 8], f32, tag="rec")
                rec_v = vap(rec, 0, [[ps_(rec), P], [8, Q], [1, 8]])
                nc.vector.reciprocal(out=rec_v, in_=den_v)

                # weighted messages, in place over the gathered h columns
                gh4 = vap(gt, 0, [[gps, P], [hs_cols, QD], [cpl, 8], [1, cpl]])
                w4 = vap(wt, 0, [[ps_(wt), P], [8, QD], [1, 8], [0, cpl]])
                nc.vector.tensor_tensor(out=gh4, in0=gh4, in1=w4, op=OP.mult)

                # aggregate over edges
                acc = out_pool.tile([P, QMAX * 128], f32, tag="acc")
                aps = ps_(acc)
                acc_v = vap(acc, 0, [[aps, P], [128, Q], [1, cout]])
                gh_cd = vap(gt, 0, [[gps, P], [hs_cols * D, Q], [1, cout], [hs_cols, D]])
                nc.vector.tensor_reduce(out=acc_v, in_=gh_cd, axis=AX.X, op=OP.add)

                # alpha normalization + bias + elu
                rec16 = vap(rec, 0, [[ps_(rec), P], [8, Q], [1, 8], [0, cpl]])
                acc_h = vap(acc, 0, [[aps, P], [128, Q], [cpl, 8], [1, cpl]])
                nc.vector.tensor_tensor(out=acc_h, in0=acc_h, in1=rec16, op=OP.mult)
                b_v = vap(B_sb[li], 0, [[ps_(B_sb[li]), P], [0, Q], [1, cout]])
                nc.vector.tensor_tensor(out=acc_v, in0=acc_v, in1=b_v, op=OP.add)
                tneg = out_pool.tile([P, QMAX * 128], f32, tag="tneg")
                tneg_v = vap(tneg, 0, [[ps_(tneg), P], [128, Q], [1, cout]])
                nc.vector.tensor_scalar(out=tneg_v, in0=acc_v, scalar1=0.0,
                                        scalar2=None, op0=OP.min)
                nc.scalar.activation(out=tneg_v, in_=tneg_v, func=AF.Exp)
                nc.vector.tensor_scalar(out=acc_v, in0=acc_v, scalar1=0.0,
                                        scalar2=-1.0, op0=OP.max, op1=OP.add)
                xe = out_pool.tile([P, QMAX * 128], bf16, tag="xe")
                xps = ps_(xe)
                xe_v = vap(xe, 0, [[xps, P], [128, Q], [1, cout]])
                nc.vector.tensor_tensor(out=xe_v, in0=acc_v, in1=tneg_v, op=OP.add)

                # per-group epilogue: transpose for next GEMM (or store x4),
                # next-layer GEMM, chunked AllGather
                for qi in range(Q):
                    g = g0 + qi
                    k = g // gpc
                    if not last:
                        tp = tp_psum.tile([P, P], bf16, tag="tp")
                        nc.tensor.transpose(
                            out=tp[0:cout, :],
                            in_=vap(xe, qi * 128, [[xps, P], [1, cout]]),
                            identity=ident[:])
                        nc.scalar.copy(out=xT_next[0:cout, g * P:(g + 1) * P],
                                       in_=tp[0:cout, :])
                        if g % gpc == 0:
                            hs_big = hs_pool.tile([P, gpc, 136], bf16, tag="hsbig")
                        ps = mm_psum.tile([P, 144], f32, tag="ps")
                        nc.tensor.matmul(out=ps[:, 0:cout_n + 16],
                                         lhsT=xT_next[0:cout, g * P:(g + 1) * P],
                                         rhs=W_sb[li + 1][:],
                                         start=True, stop=True)
                        nc.scalar.copy(out=hs_big[:, g - k * gpc, 0:cout_n + 8],
                                       in_=ps[:, 0:cout_n + 8])
                        nc.vector.tensor_copy(out=ald_nxt[:, g * 8:(g + 1) * 8],
                                              in_=ps[:, cout_n + 8:cout_n + 16])
                        if (g + 1) % gpc == 0:
                            nc.sync.dma_start(out=dram_ap(hs_sl[k], 0, R, 136),
                                              in_=hs_big[:])
                            nc.gpsimd.collective_compute(
                                "AllGather", OP.bypass, replica_groups=replica,
                                ins=[AP(hs_sl[k], 0, [[136, R], [1, 136]]).opt()],
                                outs=[AP(ntab, k * cfg.nc_cores * R * 136,
                                         [[136, cfg.nc_cores * R], [1, 136]]).opt()])
                    else:
                        nc.sync.dma_start(
                            out=dram_ap(x4_sl[k], (g - k * gpc) * P, P, 128),
                            in_=vap(xe, qi * 128, [[xps, P], [1, 128]]))
                        if (g + 1) % gpc == 0:
                            nc.gpsimd.collective_compute(
                                "AllGather", OP.bypass, replica_groups=replica,
                                ins=[AP(x4_sl[k], 0, [[128, R], [1, 128]]).opt()],
                                outs=[AP(x4_t, k * cfg.nc_cores * R * 128,
                                         [[128, cfg.nc_cores * R], [1, 128]]).opt()])

        # ---- pooling + FC + log_softmax ----
        x4_ap = x4_t.ap()
        sum_acc = const.tile([ngc, 128], f32, tag="sumacc")
        max_acc = const.tile([ngc, 128], f32, tag="maxacc")
        for ck in range(cfg.nch_g):
            git = idx_pool.tile([ngc, DGC], i32, tag="gidx")
            nc.sync.dma_start(out=git[:],
                              in_=AP(gidx, ck * DGC, [[DGP, ngc], [1, DGC]]))
            pg = pg_pool.tile([ngc, DGC * 128], bf16, tag="pgath")
            pps = ps_(pg)
            pg_v = vap(pg, 0, [[pps, ngc], [128, DGC], [1, 128]])
            for j in range(DGC):
                nc.gpsimd.indirect_dma_start(
                    out=vap(pg, j * 128, [[pps, ngc], [1, 128]]),
                    out_offset=None, in_=x4_ap,
                    in_offset=IndirectOffsetOnAxis(ap=git[:, j:j + 1], axis=0))
            mk = idx_pool.tile([ngc, DGC], bf16, tag="gmask")
            nc.sync.dma_start(out=mk[:],
                              in_=AP(gmask, ck * DGC, [[DGP, ngc], [1, DGC]]))
            # max over raw rows (pads duplicate the first node)
            mx = const.tile([ngc, 128], f32, tag="mxtmp")
            pg_cd = vap(pg, 0, [[pps, ngc], [1, 128], [128, DGC]])
            nc.vector.tensor_reduce(out=mx[:], in_=pg_cd, axis=AX.X, op=OP.max)
            # masked sum
            mk_v = vap(mk, 0, [[ps_(mk), ngc], [1, DGC], [0, 128]])
            nc.vector.tensor_tensor(out=pg_v, in0=pg_v, in1=mk_v, op=OP.mult)
            sm = const.tile([ngc, 128], f32, tag="smtmp")
            nc.vector.tensor_reduce(out=sm[:], in_=pg_cd, axis=AX.X, op=OP.add)
            if ck == 0:
                nc.vector.tensor_copy(out=sum_acc[:], in_=sm[:])
                nc.vector.tensor_copy(out=max_acc[:], in_=mx[:])
            else:
                nc.vector.tensor_tensor(out=sum_acc[:], in0=sum_acc[:],
                                        in1=sm[:], op=OP.add)
                nc.vector.tensor_tensor(out=max_acc[:], in0=max_acc[:],
                                        in1=mx[:], op=OP.max)
        nc.vector.tensor_scalar(out=sum_acc[:], in0=sum_acc[:], scalar1=ginv_sb[:],
                                scalar2=None, op0=OP.mult)
        nc.vector.tensor_scalar(out=max_acc[:], in0=max_acc[:], scalar1=gmax_sb[:],
                                scalar2=None, op0=OP.mult)

        mean_bf = const.tile([ngc, 128], bf16, tag="meanbf")
        max_bf = const.tile([ngc, 128], bf16, tag="maxbf")
        nc.vector.tensor_copy(out=mean_bf[:], in_=sum_acc[:])
        nc.vector.tensor_copy(out=max_bf[:], in_=max_acc[:])
        featT = const.tile([P, 2, ngc], bf16, tag="featT")
        tpm = tp_psum.tile([P, P], bf16, tag="tp")
        nc.tensor.transpose(out=tpm[:, 0:ngc], in_=mean_bf[:],
                            identity=ident[0:ngc, 0:ngc])
        nc.vector.tensor_copy(out=featT[:, 0, :], in_=tpm[:, 0:ngc])
        tpx = tp_psum.tile([P, P], bf16, tag="tp")
        nc.tensor.transpose(out=tpx[:, 0:ngc], in_=max_bf[:],
                            identity=ident[0:ngc, 0:ngc])
        nc.vector.tensor_copy(out=featT[:, 1, :], in_=tpx[:, 0:ngc])

        zps = mm_psum.tile([ngc, 6], f32, tag="ps", name="zps")
        nc.tensor.matmul(out=zps[:], lhsT=featT[:, 0, :], rhs=fcW_sb[:, 0, :],
                         start=True, stop=False)
        nc.tensor.matmul(out=zps[:], lhsT=featT[:, 1, :], rhs=fcW_sb[:, 1, :],
                         start=False, stop=True)
        z = const.tile([ngc, 6], f32, tag="z")
        nc.vector.tensor_tensor(out=z[:], in0=zps[:], in1=fcb_sb[:], op=OP.add)
        zmax = const.tile([ngc, 1], f32, tag="zmax")
        nc.vector.tensor_reduce(out=zmax[:], in_=z[:], axis=AX.X, op=OP.max)
        nc.vector.tensor_scalar(out=z[:], in0=z[:], scalar1=zmax[:], scalar2=None,
                                op0=OP.subtract)
        ze = const.tile([ngc, 6], f32, tag="ze")
        nc.scalar.activation(out=ze[:], in_=z[:], func=AF.Exp)
        zs = const.tile([ngc, 1], f32, tag="zs")
        nc.vector.tensor_reduce(out=zs[:], in_=ze[:], axis=AX.X, op=OP.add)
        nc.scalar.activation(out=zs[:], in_=zs[:], func=AF.Ln)
        nc.vector.tensor_scalar(out=z[:], in0=z[:], scalar1=zs[:], scalar2=None,
                                op0=OP.subtract)
        nc.sync.dma_start(out=outp.ap(), in_=z[:])

    nc.compile()
    return nc


# ---------------------------------------------------------------------------
# host inputs per core
# ---------------------------------------------------------------------------

def _make_inputs(cfg, x, weights):
    import ml_dtypes
    bf = ml_dtypes.bfloat16
    x = np.asarray(x, dtype=np.float32)
    n = cfg.n

    folded = []
    for li, (fin, cout) in enumerate(LAYERS):
        W, a_s, a_d, _b = weights[li]
        folded.append(_fold_weights(W, a_s, a_d, fin, cout).astype(bf))

    xfull = np.zeros((cfg.NP_all, LAYERS[0][0]), dtype=np.float32)
    xfull[cfg.pos_of[np.arange(n)]] = x
    xT1_full = np.ascontiguousarray(xfull.T).astype(bf)

    fcW_np, fcb_np = weights[4]
    in_maps = []
    for c in range(cfg.nc_cores):
        xo = np.zeros((cfg.NPC, LAYERS[0][0]), dtype=np.float32)
        xo[:cfg.npc_raw] = x[cfg.perm_local[c]]
        m = {
            "xT1_full": xT1_full,
            "xT1_own": np.ascontiguousarray(xo.T).astype(bf),
            "fcW": np.asarray(fcW_np, dtype=np.float32).astype(bf),
            "fcb": np.tile(np.asarray(fcb_np, dtype=np.float32).reshape(1, 6),
                           (cfg.ngc, 1)),
            "eidx": cfg.eidx[c].reshape(1, -1),
            "gidx": cfg.gidx[c].reshape(1, -1),
            "gmask": cfg.gmask[c].reshape(1, -1).astype(bf),
            "ginv": cfg.ginv[c],
            "gmax": cfg.gmax[c],
        }
        for li in range(4):
            m[f"W{li+1}p"] = folded[li]
            m[f"b{li+1}"] = np.tile(np.asarray(weights[li][3],
                                            dtype=np.float32).reshape(1, -1),
                                    (P, 1))
        in_maps.append(m)
    return in_maps


def kernel(x, edge_index, batch,
           W1, a1s, a1d, b1, W2, a2s, a2d, b2,
           W3, a3s, a3d, b3, W4, a4s, a4d, b4, fcW, fcb):
    global LAST_EXEC_NS
    from concourse.bass_utils import run_bass_kernel_spmd

    weights = [(W1, a1s, a1d, b1), (W2, a2s, a2d, b2),
               (W3, a3s, a3d, b3), (W4, a4s, a4d, b4), (fcW, fcb)]
    cfg = _prep(x, edge_index, batch, N_NODES, N_GRAPHS)
    nc = _build(cfg)
    in_maps = _make_inputs(cfg, x, weights)
    trace = os.environ.get("GAT_TRACE", "0") == "1"
    res = run_bass_kernel_spmd(nc, in_maps, core_ids=list(range(cfg.nc_cores)),
                               trace=trace)
    LAST_EXEC_NS = res.exec_time_ns
    out = np.concatenate([np.asarray(res.results[c]["out"], dtype=np.float32)
                          for c in range(cfg.nc_cores)], axis=0)
    return out
